# Initial kernel scaffold
#
# Trainium2 Bass kernel for nn_AttentionBlock (GroupNorm + single-head
# self-attention over 32x32 spatial, C=512) — data-parallel over batch:
# 8 batch elements -> 8 NeuronCores, weights replicated.
#
# Self-contained: builds the Bass module lazily, shards the full inputs,
# runs via concourse.bass_utils.run_bass_kernel_spmd, gathers the output.
import numpy as np

CH = 512          # channels
N = 1024          # spatial H*W = 32*32
P = 128           # SBUF partitions
KT = CH // P      # 4 channel tiles
MT = N // P       # 8 spatial tiles (keys)
GROUPS = 8        # groupnorm groups (64 channels each)
EPS = 1e-5
SCALE = 1.0 / np.sqrt(CH)
NCORES = 8

_CACHE = {}


def _build_bass():
    import concourse.bacc as bacc
    import concourse.tile as tile
    from concourse import mybir

    f32 = mybir.dt.float32
    f32r = mybir.dt.float32r
    Act = mybir.ActivationFunctionType
    Alu = mybir.AluOpType

    nc = bacc.Bacc("TRN2")

    x_d = nc.dram_tensor("x", [CH, N], f32, kind="ExternalInput")
    wq_d = nc.dram_tensor("wq_t", [CH, CH], f32, kind="ExternalInput")
    wk_d = nc.dram_tensor("wk_t", [CH, CH], f32, kind="ExternalInput")
    wv_d = nc.dram_tensor("wv_t", [CH, CH], f32, kind="ExternalInput")
    wp_d = nc.dram_tensor("wp_t", [CH, CH], f32, kind="ExternalInput")
    # packed per-channel vectors: cols = bq|bk|bv|bp|gnw|gnb (4 each)
    vec_d = nc.dram_tensor("vecs", [P, 32], f32, kind="ExternalInput")
    # identity (for PE transposes) | block-diag group-averaging matrix
    con_d = nc.dram_tensor("consts", [P, 2, P], f32, kind="ExternalInput")
    y_d = nc.dram_tensor("y", [CH, N], f32, kind="ExternalOutput")

    with tile.TileContext(nc) as tc:
        with (
            tc.tile_pool(name="persist", bufs=1) as persist,
            tc.tile_pool(name="work", bufs=2) as work,
            tc.tile_pool(name="small", bufs=2) as small,
            tc.tile_pool(name="ytiles", bufs=2) as ypool,
        ):
            # ---- persistent SBUF tensors ----
            x_sb = persist.tile([P, KT, N], f32, tag="x")
            n_sb = persist.tile([P, KT, N], f32r, tag="n")
            q_sb = persist.tile([P, KT, N], f32r, tag="q")
            k_sb = persist.tile([P, KT, N], f32r, tag="k")
            vT_sb = persist.tile([P, MT, CH], f32r, tag="vT")
            aT_sb = persist.tile([P, MT, N], f32r, tag="aT")
            o_sb = persist.tile([P, KT, N], f32r, tag="o")
            wq_sb = persist.tile([P, KT, CH], f32r, tag="wq")
            wk_sb = persist.tile([P, KT, CH], f32r, tag="wk")
            wv_sb = persist.tile([P, KT, CH], f32r, tag="wv")
            wp_sb = persist.tile([P, KT, CH], f32r, tag="wp")
            vec_sb = persist.tile([P, 32], f32, tag="vecs")
            ident_sb = persist.tile([P, P], f32r, tag="ident")
            avg_sb = persist.tile([P, P], f32, tag="avg")
            zero_sb = persist.tile([P, 1], f32, tag="zero")
            eps_sb = persist.tile([P, 1], f32, tag="eps")
            dummy_sb = persist.tile([P, 1], f32, tag="dummy")
            bq_sb = vec_sb[:, 0:4]
            bk_sb = vec_sb[:, 4:8]
            bv_sb = vec_sb[:, 8:12]
            bp_sb = vec_sb[:, 12:16]
            gnw_sb = vec_sb[:, 16:20]
            gnb_sb = vec_sb[:, 20:24]
            gnwn_sb = vec_sb[:, 24:28]
            gnbn_sb = vec_sb[:, 28:32]

            # constants + ACT sqrt-table preload while DMAs stream
            nc.vector.memset(zero_sb, 0.0)
            nc.vector.memset(eps_sb, EPS)
            nc.vector.memset(dummy_sb, 1.0)
            nc.scalar.activation(out=dummy_sb, in_=dummy_sb, func=Act.Sqrt,
                                 bias=zero_sb, scale=1.0)

            # ---- loads: one sync HWDGE queue, ordered by first use so
            # each transfer gets the full HBM bandwidth in sequence ----
            nc.sync.dma_start(out=x_sb[:, 0, 0:512], in_=x_d[0:P, 0:512])
            nc.sync.dma_start(out=x_sb[:, 0, 512:1024], in_=x_d[0:P, 512:1024])
            nc.sync.dma_start(out=vec_sb[:], in_=vec_d[:])
            nc.sync.dma_start(out=ident_sb[:], in_=con_d[:, 0, :].bitcast(f32r))
            nc.sync.dma_start(out=avg_sb[:], in_=con_d[:, 1, :])
            for kt in range(1, KT):
                nc.sync.dma_start(out=x_sb[:, kt, :], in_=x_d[kt * P:(kt + 1) * P, :])
            for w_sb, w_d in ((wq_sb, wq_d), (wk_sb, wk_d),
                              (wv_sb, wv_d), (wp_sb, wp_d)):
                src = w_d[:, :].rearrange("(t p) c -> p t c", p=P).bitcast(f32r)
                nc.sync.dma_start(out=w_sb[:], in_=src)

            with tc.tile_pool(name="ps_a", bufs=2, space="PSUM") as ps_a:
                # ---- GroupNorm, pipelined per channel-tile ----
                # Per-channel mean/E[x^2]: bn_stats on DVE for kt 0/1/3,
                # ACT accumulators for kt 2. Group aggregate+broadcast in a
                # single matmul with a host-built block-averaging matrix.
                # PE warmup: dense N=512 matmuls on a repeated identity
                # keep the HAM clock ramping while stats stream on DVE/ACT.
                import concourse.bass as bass_mod
                iap = ident_sb[:, :]
                ident512 = bass_mod.AP(
                    tensor=iap.tensor, offset=iap.offset,
                    ap=[iap.ap[0], [0, 4], iap.ap[1]],
                )
                warm_ps = ps_a.tile([P, 512], f32, tag="warm")

                def warm(k):
                    for _ in range(k):
                        nc.tensor.matmul(warm_ps, ident_sb[:], ident512,
                                         start=True, stop=True)

                warm(6)
                sd_last = None
                for kt in range(KT):
                    st = small.tile([P, 2], f32, tag="st")  # mean | E[x^2]
                    if kt != 2:
                        bstats = small.tile([P, 2, 6], f32, tag="bstats")
                        mv = small.tile([P, 2], f32, tag="mv")
                        nc.vector.bn_stats(out=bstats[:, 0, :], in_=x_sb[:, kt, 0:512])
                        nc.vector.bn_stats(out=bstats[:, 1, :], in_=x_sb[:, kt, 512:1024])
                        nc.vector.bn_aggr(out=mv, in_=bstats)
                        nc.vector.tensor_copy(st[:, 0:1], mv[:, 0:1])
                        nc.vector.scalar_tensor_tensor(
                            out=st[:, 1:2], in0=mv[:, 0:1], scalar=mv[:, 0:1],
                            in1=mv[:, 1:2], op0=Alu.mult, op1=Alu.add,
                        )
                    else:
                        # ACT accumulators; scale folded so accum_out is the
                        # mean (sum(x/N)) and E[x^2] (sum((x/sqrt(N))^2))
                        scratch = work.tile([P, N], f32, tag="scratch")
                        nc.scalar.activation(out=scratch, in_=x_sb[:, kt, :],
                                             func=Act.Identity, bias=zero_sb,
                                             scale=1.0 / N, accum_out=st[:, 0:1])
                        nc.scalar.activation(out=scratch, in_=x_sb[:, kt, :],
                                             func=Act.Square, bias=zero_sb,
                                             scale=1.0 / np.sqrt(N),
                                             accum_out=st[:, 1:2])

                    # group stats broadcast to all 128 partitions: one matmul
                    b_ps = ps_a.tile([P, 2], f32, tag="mm")
                    nc.tensor.matmul(b_ps, avg_sb, st, start=True, stop=True)
                    warm(5)
                    bc = small.tile([P, 2], f32, tag="bc")
                    nc.scalar.copy(bc, b_ps)
                    mean = bc[:, 0:1]
                    vneg = small.tile([P, 1], f32, tag="vneg")
                    nc.vector.scalar_tensor_tensor(
                        out=vneg, in0=mean, scalar=mean, in1=bc[:, 1:2],
                        op0=Alu.mult, op1=Alu.subtract,  # mean^2 - E[x^2]
                    )
                    var = small.tile([P, 1], f32, tag="var")
                    nc.vector.tensor_scalar_mul(var, vneg, -1.0)
                    sd = small.tile([P, 1], f32, tag="sd")
                    nc.scalar.activation(out=sd, in_=var, func=Act.Sqrt,
                                         bias=eps_sb, scale=1.0)
                    sd_last = sd
                    rstd = small.tile([P, 1], f32, tag="rstd")
                    nc.vector.reciprocal(rstd, sd)
                    gsc = small.tile([P, 1], f32, tag="gsc")
                    nc.vector.tensor_mul(gsc, rstd, gnw_sb[:, kt:kt + 1])
                    gshn = small.tile([P, 1], f32, tag="gshn")  # mean*gsc - gnb
                    nc.vector.scalar_tensor_tensor(
                        out=gshn, in0=mean, scalar=gsc, in1=gnb_sb[:, kt:kt + 1],
                        op0=Alu.mult, op1=Alu.subtract,
                    )
                    nc.vector.tensor_scalar(
                        out=n_sb[:, kt, :], in0=x_sb[:, kt, :],
                        scalar1=gsc, scalar2=gshn, op0=Alu.mult, op1=Alu.subtract,
                    )

                warm(8)

                # preload the exp table while the QKV matmuls stream
                nc.scalar.activation(out=dummy_sb, in_=n_sb[:, 3, 0:1], func=Act.Exp,
                                     bias=zero_sb, scale=1.0)

                # ---- Q projection ----
                for w_sb, b_sb, dst in ((wq_sb, bq_sb, q_sb),):
                    for dt in range(KT):
                        for nh in range(2):
                            mm = ps_a.tile([P, 512], f32, tag="mm")
                            for kt in range(KT):
                                nc.tensor.matmul(
                                    mm,
                                    w_sb[:, kt, dt * P:(dt + 1) * P],
                                    n_sb[:, kt, nh * 512:(nh + 1) * 512],
                                    start=(kt == 0), stop=(kt == KT - 1),
                                )
                            nc.scalar.activation(
                                out=dst[:, dt, nh * 512:(nh + 1) * 512], in_=mm,
                                func=Act.Identity, bias=b_sb[:, dt:dt + 1], scale=1.0,
                            )

                # ---- V transposed: vT[m, c] = sum_c' n[c', m] wv_t[c', c] ----
                # (v bias folds into the attention output: attn rows sum to 1)
                for mt in range(MT):
                    mm = ps_a.tile([P, 512], f32, tag="mm")
                    for kt in range(KT):
                        nc.tensor.matmul(
                            mm,
                            n_sb[:, kt, mt * P:(mt + 1) * P],
                            wv_sb[:, kt, :],
                            start=(kt == 0), stop=(kt == KT - 1),
                        )
                    nc.scalar.copy(vT_sb[:, mt, :], mm)

                # ---- K projection ----
                for w_sb, b_sb, dst in ((wk_sb, bk_sb, k_sb),):
                    for dt in range(KT):
                        for nh in range(2):
                            mm = ps_a.tile([P, 512], f32, tag="mm")
                            for kt in range(KT):
                                nc.tensor.matmul(
                                    mm,
                                    w_sb[:, kt, dt * P:(dt + 1) * P],
                                    n_sb[:, kt, nh * 512:(nh + 1) * 512],
                                    start=(kt == 0), stop=(kt == KT - 1),
                                )
                            nc.scalar.activation(
                                out=dst[:, dt, nh * 512:(nh + 1) * 512], in_=mm,
                                func=Act.Identity, bias=b_sb[:, dt:dt + 1], scale=1.0,
                            )

            # ---- attention, software-pipelined over 128-query blocks ----
            # scores -> exp (row sums via ACT accumulator; max-subtraction
            # dropped: |s*scale| < ~2 so exp is safe and softmax is
            # shift-invariant) -> normalize in place -> PE transpose-mode
            # (f32r: 1.5 cyc/row). Two score blocks run ahead of the
            # softmax/transpose of the previous block to keep PE dense.
            with (
                tc.tile_pool(name="ps_s", bufs=3, space="PSUM") as ps_s,
                tc.tile_pool(name="ps_t", bufs=2, space="PSUM") as ps_t,
            ):
                def emit_scores(nb):
                    s_ps = ps_s.tile([P, N], f32, tag="s", name=f"s{nb}")
                    for mh in range(2):
                        for kt in range(KT):
                            nc.tensor.matmul(
                                s_ps[:, mh * 512:(mh + 1) * 512],
                                q_sb[:, kt, nb * P:(nb + 1) * P],
                                k_sb[:, kt, mh * 512:(mh + 1) * 512],
                                start=(kt == 0), stop=(kt == KT - 1),
                            )
                    return s_ps

                def emit_softmax(nb, s_ps):
                    p_exp = work.tile([P, N], f32r, tag="pexp", name=f"pexp{nb}")
                    sumexp = small.tile([P, 1], f32, tag="sumexp")
                    nc.scalar.activation(out=p_exp, in_=s_ps, func=Act.Exp,
                                         bias=zero_sb, scale=SCALE,
                                         accum_out=sumexp)
                    rsum = small.tile([P, 1], f32, tag="rsum")
                    nc.vector.reciprocal(rsum, sumexp)
                    nc.vector.tensor_scalar_mul(p_exp, p_exp, rsum)
                    return p_exp

                def emit_transposes(nb, p_exp):
                    for mg in range(2):
                        t_ps = ps_t.tile([P, 512], f32r, tag="t")
                        for mi in range(4):
                            mt = mg * 4 + mi
                            nc.tensor.transpose(
                                t_ps[:, mi * P:(mi + 1) * P],
                                p_exp[:, mt * P:(mt + 1) * P],
                                ident_sb[:],
                            )
                        nc.vector.tensor_copy(
                            aT_sb[:, mg * 4:(mg + 1) * 4, nb * P:(nb + 1) * P],
                            t_ps.rearrange("p (g c) -> p g c", g=4),
                        )

                pipeline = []
                for nb in range(MT):
                    s_ps = emit_scores(nb)
                    pe = emit_softmax(nb, s_ps)
                    pipeline.append((nb, pe))
                    if len(pipeline) > 2:
                        emit_transposes(*pipeline.pop(0))
                for item in pipeline:
                    emit_transposes(*item)

            with tc.tile_pool(name="ps_b", bufs=4, space="PSUM") as ps_b:
                # ---- out[c, n] = sum_m vT[m, c] attnT[m, n] (+ bv, folded) ----
                for ct in range(KT):
                    for nh in range(2):
                        mm = ps_b.tile([P, 512], f32, tag="mm")
                        for mt in range(MT):
                            nc.tensor.matmul(
                                mm,
                                vT_sb[:, mt, ct * P:(ct + 1) * P],
                                aT_sb[:, mt, nh * 512:(nh + 1) * 512],
                                start=(mt == 0), stop=(mt == MT - 1),
                            )
                        nc.scalar.activation(
                            out=o_sb[:, ct, nh * 512:(nh + 1) * 512], in_=mm,
                            func=Act.Identity, bias=bv_sb[:, ct:ct + 1], scale=1.0,
                        )

                # ---- final projection + bias + residual, stream out ----
                for dt in range(KT):
                    y_sb = ypool.tile([P, N], f32, tag="y")
                    for nh in range(2):
                        mm = ps_b.tile([P, 512], f32, tag="mm")
                        for kt in range(KT):
                            nc.tensor.matmul(
                                mm,
                                wp_sb[:, kt, dt * P:(dt + 1) * P],
                                o_sb[:, kt, nh * 512:(nh + 1) * 512],
                                start=(kt == 0), stop=(kt == KT - 1),
                            )
                        nc.vector.scalar_tensor_tensor(
                            out=y_sb[:, nh * 512:(nh + 1) * 512], in0=mm,
                            scalar=bp_sb[:, dt:dt + 1],
                            in1=x_sb[:, dt, nh * 512:(nh + 1) * 512],
                            op0=Alu.add, op1=Alu.add,
                        )
                        nc.sync.dma_start(
                            out=y_d[dt * P:(dt + 1) * P, nh * 512:(nh + 1) * 512],
                            in_=y_sb[:, nh * 512:(nh + 1) * 512],
                        )

    nc.finalize()
    return nc


def _get_nc():
    if "nc" not in _CACHE:
        _CACHE["nc"] = _build_bass()
    return _CACHE["nc"]


def _make_in_maps(x, gn_w, gn_b, q_w, q_b, k_w, k_b, v_w, v_b, p_w, p_b):
    x = np.asarray(x, np.float32)
    B = x.shape[0]
    assert x.shape == (B, CH, 32, 32) and B == NCORES

    def pc(vec):  # [512] -> [128, 4] with c = t*128 + p
        return np.asarray(vec, np.float32).reshape(KT, P).T

    vecs = np.concatenate(
        [pc(q_b), pc(k_b), pc(v_b), pc(p_b), pc(gn_w), pc(gn_b),
         -pc(gn_w), -pc(gn_b)], axis=1
    )
    # identity + block-diagonal 64-channel averaging matrix, stacked
    avg = np.kron(np.eye(2, dtype=np.float32),
                  np.full((64, 64), 1.0 / 64, np.float32))
    consts = np.stack([np.eye(P, dtype=np.float32), avg], axis=1)
    shared = {
        "wq_t": np.ascontiguousarray(np.asarray(q_w, np.float32).T),
        "wk_t": np.ascontiguousarray(np.asarray(k_w, np.float32).T),
        "wv_t": np.ascontiguousarray(np.asarray(v_w, np.float32).T),
        "wp_t": np.ascontiguousarray(np.asarray(p_w, np.float32).T),
        "vecs": np.ascontiguousarray(vecs),
        "consts": np.ascontiguousarray(consts),
    }
    return [
        dict(shared, x=np.ascontiguousarray(x[b].reshape(CH, N)))
        for b in range(B)
    ]


def _run(in_maps, **kwargs):
    from concourse.bass_utils import run_bass_kernel_spmd
    return run_bass_kernel_spmd(_get_nc(), in_maps, core_ids=list(range(NCORES)), **kwargs)


def kernel(**inputs):
    in_maps = _make_in_maps(**inputs)
    res = _run(in_maps)
    out = np.stack([r["y"].reshape(CH, 32, 32) for r in res.results], axis=0)
    return out.astype(np.float32)



# revision 1
# speedup vs baseline: 1.5454x; 1.5454x over previous
# Trainium2 Bass kernel for nn_AttentionBlock (GroupNorm + single-head
# self-attention over 32x32 spatial, C=512) — data-parallel over batch:
# 8 batch elements -> 8 NeuronCores, weights replicated.
#
# Self-contained: builds the Bass module lazily, shards the full inputs,
# runs via concourse.bass_utils.run_bass_kernel_spmd, gathers the output.
import numpy as np

CH = 512          # channels
N = 1024          # spatial H*W = 32*32
P = 128           # SBUF partitions
KT = CH // P      # 4 channel tiles
MT = N // P       # 8 spatial tiles (keys)
GROUPS = 8        # groupnorm groups (64 channels each)
EPS = 1e-5
SCALE = 1.0 / np.sqrt(CH)
NCORES = 8

_CACHE = {}


def _build_bass():
    import concourse.bacc as bacc
    import concourse.tile as tile
    from concourse import mybir

    f32 = mybir.dt.float32
    f32r = mybir.dt.float32r
    Act = mybir.ActivationFunctionType
    Alu = mybir.AluOpType

    nc = bacc.Bacc("TRN2")

    x_d = nc.dram_tensor("x", [CH, N], f32, kind="ExternalInput")
    wq_d = nc.dram_tensor("wq_t", [CH, CH], f32, kind="ExternalInput")
    wk_d = nc.dram_tensor("wk_t", [CH, CH], f32, kind="ExternalInput")
    wv_d = nc.dram_tensor("wv_t", [CH, CH], f32, kind="ExternalInput")
    wp_d = nc.dram_tensor("wp_t", [CH, CH], f32, kind="ExternalInput")
    # packed per-channel vectors: cols = bq|bk|bv|bp|gnw|gnb (4 each)
    vec_d = nc.dram_tensor("vecs", [P, 32], f32, kind="ExternalInput")
    # identity (for PE transposes) | block-diag group-averaging matrix
    con_d = nc.dram_tensor("consts", [P, 2, P], f32, kind="ExternalInput")
    y_d = nc.dram_tensor("y", [CH, N], f32, kind="ExternalOutput")

    with tile.TileContext(nc) as tc:
        with (
            tc.tile_pool(name="persist", bufs=1) as persist,
            tc.tile_pool(name="work", bufs=2) as work,
            tc.tile_pool(name="small", bufs=2) as small,
            tc.tile_pool(name="ytiles", bufs=2) as ypool,
        ):
            # ---- persistent SBUF tensors ----
            x_sb = persist.tile([P, KT, N], f32, tag="x")
            n_sb = persist.tile([P, KT, N], f32r, tag="n")
            q_sb = persist.tile([P, KT, N], f32r, tag="q")
            k_sb = persist.tile([P, KT, N], f32r, tag="k")
            vT_sb = persist.tile([P, MT, CH], f32r, tag="vT")
            aT_sb = persist.tile([P, MT, N], f32r, tag="aT")
            o_sb = persist.tile([P, KT, N], f32r, tag="o")
            wq_sb = persist.tile([P, KT, CH], f32r, tag="wq")
            wk_sb = persist.tile([P, KT, CH], f32r, tag="wk")
            wv_sb = persist.tile([P, KT, CH], f32r, tag="wv")
            wp_sb = persist.tile([P, KT, CH], f32r, tag="wp")
            vec_sb = persist.tile([P, 32], f32, tag="vecs")
            ident_sb = persist.tile([P, P], f32r, tag="ident")
            avg_sb = persist.tile([P, P], f32, tag="avg")
            zero_sb = persist.tile([P, 1], f32, tag="zero")
            eps_sb = persist.tile([P, 1], f32, tag="eps")
            dummy_sb = persist.tile([P, 1], f32, tag="dummy")
            bq_sb = vec_sb[:, 0:4]
            bk_sb = vec_sb[:, 4:8]
            bv_sb = vec_sb[:, 8:12]
            bp_sb = vec_sb[:, 12:16]
            gnw_sb = vec_sb[:, 16:20]
            gnb_sb = vec_sb[:, 20:24]
            gnwn_sb = vec_sb[:, 24:28]
            gnbn_sb = vec_sb[:, 28:32]

            # constants + ACT sqrt-table preload while DMAs stream
            nc.vector.memset(zero_sb, 0.0)
            nc.vector.memset(eps_sb, EPS)
            nc.vector.memset(dummy_sb, 1.0)
            nc.scalar.activation(out=dummy_sb, in_=dummy_sb, func=Act.Sqrt,
                                 bias=zero_sb, scale=1.0)

            # ---- loads: one sync HWDGE queue, ordered by first use so
            # each transfer gets the full HBM bandwidth in sequence ----
            nc.sync.dma_start(out=x_sb[:, 0, 0:512], in_=x_d[0:P, 0:512])
            nc.sync.dma_start(out=x_sb[:, 0, 512:1024], in_=x_d[0:P, 512:1024])
            nc.sync.dma_start(out=vec_sb[:], in_=vec_d[:])
            nc.sync.dma_start(out=ident_sb[:], in_=con_d[:, 0, :].bitcast(f32r))
            nc.sync.dma_start(out=avg_sb[:], in_=con_d[:, 1, :])
            for kt in range(1, KT):
                nc.sync.dma_start(out=x_sb[:, kt, :], in_=x_d[kt * P:(kt + 1) * P, :])
            for w_sb, w_d in ((wq_sb, wq_d), (wk_sb, wk_d),
                              (wv_sb, wv_d), (wp_sb, wp_d)):
                src = w_d[:, :].rearrange("(t p) c -> p t c", p=P).bitcast(f32r)
                nc.sync.dma_start(out=w_sb[:], in_=src)

            with tc.tile_pool(name="ps_a", bufs=2, space="PSUM") as ps_a:
                # ---- GroupNorm, pipelined per channel-tile ----
                # Per-channel mean/E[x^2]: bn_stats on DVE for kt 0/1/3,
                # ACT accumulators for kt 2. Group aggregate+broadcast in a
                # single matmul with a host-built block-averaging matrix.
                # PE warmup: dense N=512 matmuls on a repeated identity
                # keep the HAM clock ramping while stats stream on DVE/ACT.
                import concourse.bass as bass_mod
                iap = ident_sb[:, :]
                ident512 = bass_mod.AP(
                    tensor=iap.tensor, offset=iap.offset,
                    ap=[iap.ap[0], [0, 4], iap.ap[1]],
                )
                warm_ps = ps_a.tile([P, 512], f32, tag="warm")

                def warm(k):
                    for _ in range(k):
                        nc.tensor.matmul(warm_ps, ident_sb[:], ident512,
                                         start=True, stop=True)

                warm(6)
                sd_last = None
                for kt in range(KT):
                    st = small.tile([P, 2], f32, tag="st")  # mean | E[x^2]
                    if kt != 2:
                        bstats = small.tile([P, 2, 6], f32, tag="bstats")
                        mv = small.tile([P, 2], f32, tag="mv")
                        nc.vector.bn_stats(out=bstats[:, 0, :], in_=x_sb[:, kt, 0:512])
                        nc.vector.bn_stats(out=bstats[:, 1, :], in_=x_sb[:, kt, 512:1024])
                        nc.vector.bn_aggr(out=mv, in_=bstats)
                        nc.vector.tensor_copy(st[:, 0:1], mv[:, 0:1])
                        nc.vector.scalar_tensor_tensor(
                            out=st[:, 1:2], in0=mv[:, 0:1], scalar=mv[:, 0:1],
                            in1=mv[:, 1:2], op0=Alu.mult, op1=Alu.add,
                        )
                    else:
                        # ACT accumulators; scale folded so accum_out is the
                        # mean (sum(x/N)) and E[x^2] (sum((x/sqrt(N))^2))
                        scratch = work.tile([P, N], f32, tag="scratch")
                        nc.scalar.activation(out=scratch, in_=x_sb[:, kt, :],
                                             func=Act.Identity, bias=zero_sb,
                                             scale=1.0 / N, accum_out=st[:, 0:1])
                        nc.scalar.activation(out=scratch, in_=x_sb[:, kt, :],
                                             func=Act.Square, bias=zero_sb,
                                             scale=1.0 / np.sqrt(N),
                                             accum_out=st[:, 1:2])

                    # group stats broadcast to all 128 partitions: one matmul
                    b_ps = ps_a.tile([P, 2], f32, tag="mm")
                    nc.tensor.matmul(b_ps, avg_sb, st, start=True, stop=True)
                    warm(5)
                    bc = small.tile([P, 2], f32, tag="bc")
                    nc.scalar.copy(bc, b_ps)
                    mean = bc[:, 0:1]
                    vneg = small.tile([P, 1], f32, tag="vneg")
                    nc.vector.scalar_tensor_tensor(
                        out=vneg, in0=mean, scalar=mean, in1=bc[:, 1:2],
                        op0=Alu.mult, op1=Alu.subtract,  # mean^2 - E[x^2]
                    )
                    var = small.tile([P, 1], f32, tag="var")
                    nc.vector.tensor_scalar_mul(var, vneg, -1.0)
                    sd = small.tile([P, 1], f32, tag="sd")
                    nc.scalar.activation(out=sd, in_=var, func=Act.Sqrt,
                                         bias=eps_sb, scale=1.0)
                    sd_last = sd
                    rstd = small.tile([P, 1], f32, tag="rstd")
                    nc.vector.reciprocal(rstd, sd)
                    gsc = small.tile([P, 1], f32, tag="gsc")
                    nc.vector.tensor_mul(gsc, rstd, gnw_sb[:, kt:kt + 1])
                    gshn = small.tile([P, 1], f32, tag="gshn")  # mean*gsc - gnb
                    nc.vector.scalar_tensor_tensor(
                        out=gshn, in0=mean, scalar=gsc, in1=gnb_sb[:, kt:kt + 1],
                        op0=Alu.mult, op1=Alu.subtract,
                    )
                    nc.vector.tensor_scalar(
                        out=n_sb[:, kt, :], in0=x_sb[:, kt, :],
                        scalar1=gsc, scalar2=gshn, op0=Alu.mult, op1=Alu.subtract,
                    )

                warm(8)

                # preload the exp table while the QKV matmuls stream
                nc.scalar.activation(out=dummy_sb, in_=n_sb[:, 3, 0:1], func=Act.Exp,
                                     bias=zero_sb, scale=1.0)

                # ---- Q projection ----
                for w_sb, b_sb, dst in ((wq_sb, bq_sb, q_sb),):
                    for dt in range(KT):
                        for nh in range(2):
                            mm = ps_a.tile([P, 512], f32, tag="mm")
                            for kt in range(KT):
                                nc.tensor.matmul(
                                    mm,
                                    w_sb[:, kt, dt * P:(dt + 1) * P],
                                    n_sb[:, kt, nh * 512:(nh + 1) * 512],
                                    start=(kt == 0), stop=(kt == KT - 1),
                                )
                            nc.scalar.activation(
                                out=dst[:, dt, nh * 512:(nh + 1) * 512], in_=mm,
                                func=Act.Identity, bias=b_sb[:, dt:dt + 1], scale=1.0,
                            )

                # ---- V transposed: vT[m, c] = sum_c' n[c', m] wv_t[c', c] ----
                # (v bias folds into the attention output: attn rows sum to 1)
                for mt in range(MT):
                    mm = ps_a.tile([P, 512], f32, tag="mm")
                    for kt in range(KT):
                        nc.tensor.matmul(
                            mm,
                            n_sb[:, kt, mt * P:(mt + 1) * P],
                            wv_sb[:, kt, :],
                            start=(kt == 0), stop=(kt == KT - 1),
                        )
                    nc.scalar.copy(vT_sb[:, mt, :], mm)

                # ---- K projection ----
                for w_sb, b_sb, dst in ((wk_sb, bk_sb, k_sb),):
                    for dt in range(KT):
                        for nh in range(2):
                            mm = ps_a.tile([P, 512], f32, tag="mm")
                            for kt in range(KT):
                                nc.tensor.matmul(
                                    mm,
                                    w_sb[:, kt, dt * P:(dt + 1) * P],
                                    n_sb[:, kt, nh * 512:(nh + 1) * 512],
                                    start=(kt == 0), stop=(kt == KT - 1),
                                )
                            nc.scalar.activation(
                                out=dst[:, dt, nh * 512:(nh + 1) * 512], in_=mm,
                                func=Act.Identity, bias=b_sb[:, dt:dt + 1], scale=1.0,
                            )

            # ---- attention, software-pipelined over 128-query blocks ----
            # scores -> exp (row sums via ACT accumulator; max-subtraction
            # dropped: |s*scale| < ~2 so exp is safe and softmax is
            # shift-invariant) -> normalize in place -> PE transpose-mode
            # (f32r: 1.5 cyc/row). Two score blocks run ahead of the
            # softmax/transpose of the previous block to keep PE dense.
            with (
                tc.tile_pool(name="ps_s", bufs=3, space="PSUM") as ps_s,
                tc.tile_pool(name="ps_t", bufs=2, space="PSUM") as ps_t,
            ):
                def emit_scores(nb):
                    s_ps = ps_s.tile([P, N], f32, tag="s", name=f"s{nb}")
                    for mh in range(2):
                        for kt in range(KT):
                            nc.tensor.matmul(
                                s_ps[:, mh * 512:(mh + 1) * 512],
                                q_sb[:, kt, nb * P:(nb + 1) * P],
                                k_sb[:, kt, mh * 512:(mh + 1) * 512],
                                start=(kt == 0), stop=(kt == KT - 1),
                            )
                    return s_ps

                def emit_softmax(nb, s_ps):
                    p_exp = work.tile([P, N], f32r, tag="pexp", name=f"pexp{nb}")
                    sumexp = small.tile([P, 1], f32, tag="sumexp")
                    nc.scalar.activation(out=p_exp, in_=s_ps, func=Act.Exp,
                                         bias=zero_sb, scale=SCALE,
                                         accum_out=sumexp)
                    rsum = small.tile([P, 1], f32, tag="rsum")
                    nc.vector.reciprocal(rsum, sumexp)
                    nc.vector.tensor_scalar_mul(p_exp, p_exp, rsum)
                    return p_exp

                def emit_transposes(nb, p_exp):
                    for mg in range(2):
                        t_ps = ps_t.tile([P, 512], f32r, tag="t")
                        for mi in range(4):
                            mt = mg * 4 + mi
                            nc.tensor.transpose(
                                t_ps[:, mi * P:(mi + 1) * P],
                                p_exp[:, mt * P:(mt + 1) * P],
                                ident_sb[:],
                            )
                        nc.vector.tensor_copy(
                            aT_sb[:, mg * 4:(mg + 1) * 4, nb * P:(nb + 1) * P],
                            t_ps.rearrange("p (g c) -> p g c", g=4),
                        )

                pipeline = []
                for nb in range(MT):
                    s_ps = emit_scores(nb)
                    pe = emit_softmax(nb, s_ps)
                    pipeline.append((nb, pe))
                    if len(pipeline) > 2:
                        emit_transposes(*pipeline.pop(0))
                for item in pipeline:
                    emit_transposes(*item)

            with tc.tile_pool(name="ps_b", bufs=4, space="PSUM") as ps_b:
                # ---- out[c, n] = sum_m vT[m, c] attnT[m, n] (+ bv, folded) ----
                for ct in range(KT):
                    for nh in range(2):
                        mm = ps_b.tile([P, 512], f32, tag="mm")
                        for mt in range(MT):
                            nc.tensor.matmul(
                                mm,
                                vT_sb[:, mt, ct * P:(ct + 1) * P],
                                aT_sb[:, mt, nh * 512:(nh + 1) * 512],
                                start=(mt == 0), stop=(mt == MT - 1),
                            )
                        nc.scalar.activation(
                            out=o_sb[:, ct, nh * 512:(nh + 1) * 512], in_=mm,
                            func=Act.Identity, bias=bv_sb[:, ct:ct + 1], scale=1.0,
                        )

                # ---- final projection + bias + residual, stream out ----
                for dt in range(KT):
                    y_sb = ypool.tile([P, N], f32, tag="y")
                    for nh in range(2):
                        mm = ps_b.tile([P, 512], f32, tag="mm")
                        for kt in range(KT):
                            nc.tensor.matmul(
                                mm,
                                wp_sb[:, kt, dt * P:(dt + 1) * P],
                                o_sb[:, kt, nh * 512:(nh + 1) * 512],
                                start=(kt == 0), stop=(kt == KT - 1),
                            )
                        nc.vector.scalar_tensor_tensor(
                            out=y_sb[:, nh * 512:(nh + 1) * 512], in0=mm,
                            scalar=bp_sb[:, dt:dt + 1],
                            in1=x_sb[:, dt, nh * 512:(nh + 1) * 512],
                            op0=Alu.add, op1=Alu.add,
                        )
                        nc.sync.dma_start(
                            out=y_d[dt * P:(dt + 1) * P, nh * 512:(nh + 1) * 512],
                            in_=y_sb[:, nh * 512:(nh + 1) * 512],
                        )

    nc.finalize()
    return nc


def _get_nc():
    if "nc" not in _CACHE:
        _CACHE["nc"] = _build_bass()
    return _CACHE["nc"]


def _make_in_maps(x, gn_w, gn_b, q_w, q_b, k_w, k_b, v_w, v_b, p_w, p_b):
    x = np.asarray(x, np.float32)
    B = x.shape[0]
    assert x.shape == (B, CH, 32, 32) and B == NCORES

    def pc(vec):  # [512] -> [128, 4] with c = t*128 + p
        return np.asarray(vec, np.float32).reshape(KT, P).T

    vecs = np.concatenate(
        [pc(q_b), pc(k_b), pc(v_b), pc(p_b), pc(gn_w), pc(gn_b),
         -pc(gn_w), -pc(gn_b)], axis=1
    )
    # identity + block-diagonal 64-channel averaging matrix, stacked
    avg = np.kron(np.eye(2, dtype=np.float32),
                  np.full((64, 64), 1.0 / 64, np.float32))
    consts = np.stack([np.eye(P, dtype=np.float32), avg], axis=1)
    shared = {
        "wq_t": np.ascontiguousarray(np.asarray(q_w, np.float32).T),
        "wk_t": np.ascontiguousarray(np.asarray(k_w, np.float32).T),
        "wv_t": np.ascontiguousarray(np.asarray(v_w, np.float32).T),
        "wp_t": np.ascontiguousarray(np.asarray(p_w, np.float32).T),
        "vecs": np.ascontiguousarray(vecs),
        "consts": np.ascontiguousarray(consts),
    }
    return [
        dict(shared, x=np.ascontiguousarray(x[b].reshape(CH, N)))
        for b in range(B)
    ]


def _run(in_maps, **kwargs):
    from concourse.bass_utils import run_bass_kernel_spmd
    return run_bass_kernel_spmd(_get_nc(), in_maps, core_ids=list(range(NCORES)), **kwargs)


def kernel(**inputs):
    in_maps = _make_in_maps(**inputs)
    res = _run(in_maps)
    out = np.stack([r["y"].reshape(CH, 32, 32) for r in res.results], axis=0)
    return out.astype(np.float32)

